# revision 1
# baseline (speedup 1.0000x reference)
"""MMoE layer kernel for 8 Trainium2 NeuronCores.

Reference math (B=4096, D=1024, H1=2048, H2=1024, E=7 experts, NS=7 scenes):
  h        = relu(einsum('bd,edh', x, W1) + b1)           # [B,E,H1]
  eo       = relu(einsum('beh,eho', h, W2) + b2)          # [B,E,H2]
  xc       = concat(x, scene_emb[scene])                  # [B, D+16]
  G        = softmax over s of einsum('bd,sde', xc, S)    # [B,E,NS] (after transpose)
  q        = mean_s log(G*7)                              # [B,E]
  score1   = logG[b, e, scene_b]
  select   = drop expert e iff e == argmin_e score1 == argmin_e q
  gate     = softmax_e(G[b,e,scene_b]) * select
  out      = einsum('be,beo', gate, eo); output = stack([out, out])

Sharding: data-parallel over batch (512 rows/core), weights replicated.
Expert MLP matmuls run in bf16 (fp32 accumulation in PSUM); all routing
math stays fp32 so the argmin/select decisions are bit-stable.

Device decomposition of the routing (no cross-partition broadcasts):
  Gpre[b, e*7+s] = x[b] @ Sflat + SE_table[scene_b]   (SE_table = scene_emb @ S[:,D:,:])
  Z = sum_s exp(Gpre); logZ = ln Z; SG = sum_s Gpre
  q      = SG/7 - logZ            (+const, argmin only)
  score1 = sum_s Gpre*onehot_s(scene) - logZ
  gate0  = softmax_e(exp(score1)) (logits in (0,1): no max-subtract needed)
  sel    = 1 - ismin(score1)*ismin(q)
  gate   = gate0 * sel
"""

import sys

if "/opt/trn_rl_repo" not in sys.path:
    sys.path.insert(0, "/opt/trn_rl_repo")

from contextlib import ExitStack

import ml_dtypes
import numpy as np

import concourse.bass as bass
import concourse.tile as tile
from concourse import bacc, mybir
from concourse.bass_utils import run_bass_kernel_spmd

F32 = mybir.dt.float32
BF16 = mybir.dt.bfloat16
AF = mybir.ActivationFunctionType
ALU = mybir.AluOpType
AX = mybir.AxisListType

N_CORES = 8
B, D, H1, H2, E, NS, T = 4096, 1024, 2048, 1024, 7, 7, 2
BL = B // N_CORES          # 512 rows per core
NB = BL // 128             # 4 batch tiles
KT1 = D // 128             # 8  k-tiles, layer 1
MT1 = H1 // 128            # 16 m-tiles, layer 1
KT2 = H1 // 128            # 16 k-tiles, layer 2
NO = H2 // 512             # 2  512-wide out column blocks
EN = E * NS                # 49
NP_BF16 = np.dtype(ml_dtypes.bfloat16)


def _emit_kernel(tc, aps, has_b1, has_b2):
    nc = tc.nc
    ctx = ExitStack()
    with ctx:
        # Pool stack order matters: the expert-weight pools are allocated
        # BEFORE the routing pool so they never reuse the routing pool's
        # released SBUF addresses — otherwise Tile serializes the first
        # weight DMAs behind every routing matmul (measured 13µs PE stall).
        consts = ctx.enter_context(tc.tile_pool(name="consts", bufs=1))
        w1pool = ctx.enter_context(tc.tile_pool(name="w1", bufs=2))
        w2pool = ctx.enter_context(tc.tile_pool(name="w2", bufs=1))
        htpool = ctx.enter_context(tc.tile_pool(name="ht", bufs=1))
        tmppool = ctx.enter_context(tc.tile_pool(name="tmp", bufs=3))
        l1ps = ctx.enter_context(tc.tile_pool(name="l1ps", bufs=4, space="PSUM"))
        l2ps = ctx.enter_context(tc.tile_pool(name="l2ps", bufs=4, space="PSUM"))
        rpool = tc.alloc_tile_pool(name="routing", bufs=1)

        # ---- PE warm-up: dummy matmuls from memset tiles (no input deps)
        # fill the ~10µs DMA boot window and flip the HAM clock gate to 8/8
        # before the fp32 routing matmuls run. ----------------------------
        warm_sb = rpool.tile([128, 512], BF16)
        nc.vector.memset(warm_sb[:, :], 0.0)
        warm_ps = l1ps.tile([128, 512], F32, tag="ps1", name="warm_ps")
        for _ in range(40):
            nc.tensor.matmul(
                warm_ps[:, :], lhsT=warm_sb[:, 0:128], rhs=warm_sb[:, :],
                start=True, stop=True,
            )

        # ---- routing inputs lead the sync queue (sflat, then xT quarters
        # so the kt-major routing matmuls start on the first quarter);
        # small constants follow — they're needed only mid-routing. --------
        sflat_sb = rpool.tile([128, KT1, EN], F32)
        nc.sync.dma_start(sflat_sb[:, :, :], aps["sflat"].rearrange("(t p) j -> p t j", p=128))
        xt_sb = rpool.tile([128, KT1, BL], F32)
        xt_src = aps["xT"].rearrange("(t p) b -> p t b", p=128)
        for qq in range(4):
            nc.sync.dma_start(
                xt_sb[:, 2 * qq : 2 * qq + 2, :], xt_src[:, 2 * qq : 2 * qq + 2, :]
            )
        sett_sb = rpool.tile([10, EN], F32)
        nc.sync.dma_start(sett_sb[:, :], aps["sett"][:, :])
        scolr_sb = rpool.tile([128, NB * EN], F32)
        nc.sync.dma_start(scolr_sb[:, :], aps["scol_rep"][:, :])
        srow10_sb = rpool.tile([10, BL], F32)
        nc.sync.dma_start(srow10_sb[:, :], aps["srow"].to_broadcast((10, BL)))
        io7_sb = rpool.tile([128, NB * EN], F32)
        nc.sync.dma_start(io7_sb[:, :], aps["iota7"].to_broadcast((128, NB * EN)))
        io10_sb = rpool.tile([10, 1], F32)
        nc.sync.dma_start(io10_sb[:, :], aps["iota10"][:, :])

        # ---- persistent SBUF state (xtb queued right after xT: layer 1 of
        # expert 0 is gated on it, so it must precede the weight stream) ---
        xtb_sb = consts.tile([128, KT1, BL], BF16)
        nc.sync.dma_start(xtb_sb[:, :, :], aps["xTb"].rearrange("(t p) b -> p t b", p=128))
        gate_sb = consts.tile([128, NB, E], F32)
        acc_sb = consts.tile([128, NB, H2], F32)
        if has_b1:
            b1_sb = consts.tile([128, E * MT1], F32)
            nc.sync.dma_start(b1_sb[:, :], aps["b1t"][:, :])
        if has_b2:
            b2_sb = consts.tile([1, E * H2], BF16)
            nc.sync.dma_start(b2_sb[:, :], aps["b2f"][:, :])
            ones_sb = consts.tile([1, 128], BF16)
            nc.vector.memset(ones_sb[:, :], 1.0)

        # onehot over embedding rows, [10, BL]: onehot[r, b] = (scene[b] == r)
        onehot_sb = rpool.tile([10, BL], F32)
        nc.vector.tensor_scalar(
            out=onehot_sb[:, :], in0=srow10_sb[:, :],
            scalar1=io10_sb[:, 0:1], scalar2=None, op0=ALU.is_equal,
        )

        # kt-major so the matmuls consume xT k-slices as they land; the four
        # b-tiles' PSUM groups live in the (not-yet-used) L1 pool slots.
        psr = []
        for t in range(NB):
            psr_t = l1ps.tile([128, EN], F32, tag="ps1", name=f"psr{t}")
            psr.append(psr_t)
        for kt in range(KT1):
            for t in range(NB):
                nc.tensor.matmul(
                    psr[t][:, :],
                    lhsT=xt_sb[:, kt, bass.ts(t, 128)],
                    rhs=sflat_sb[:, kt, :],
                    start=(kt == 0), stop=False,
                )
        gp = rpool.tile([128, NB * EN], F32)  # all 4 b-tiles side by side
        for t in range(NB):
            nc.tensor.matmul(
                psr[t][:, :],
                lhsT=onehot_sb[:, bass.ts(t, 128)],
                rhs=sett_sb[:, :],
                start=False, stop=True,
            )
            nc.scalar.copy(gp[:, bass.ts(t, EN)], psr[t][:, :])

        def routing_chain():
            """Gate computation, fused over all 4 b-tiles ([128, 4*49]).

            Emitted AFTER layer 1 of expert 0: the scalar engine's queue is
            strict FIFO, so emitting this serial chain before the L1 PSUM
            evacuations would block them (and stall the PE on PSUM slots).
            The gate is only consumed by expert 0's layer-2 evacuation.
            """
            NE = NB * E  # 28
            gp4 = gp.rearrange("p (t e s) -> p (t e) s", s=NS, e=E)
            eex = rpool.tile([128, NB * EN], F32)
            nc.scalar.activation(eex[:, :], gp[:, :], AF.Exp)
            z = rpool.tile([128, NE], F32)
            nc.vector.tensor_reduce(out=z[:, :], in_=eex.rearrange("p (t e s) -> p (t e) s", s=NS, e=E), axis=AX.X, op=ALU.add)
            logz = rpool.tile([128, NE], F32)
            nc.scalar.activation(logz[:, :], z[:, :], AF.Ln)
            sg = rpool.tile([128, NE], F32)
            nc.vector.tensor_reduce(out=sg[:, :], in_=gp4, axis=AX.X, op=ALU.add)
            q = rpool.tile([128, NE], F32)
            nc.vector.scalar_tensor_tensor(
                out=q[:, :], in0=sg[:, :], scalar=1.0 / NS, in1=logz[:, :],
                op0=ALU.mult, op1=ALU.subtract,
            )
            oh = rpool.tile([128, NB * EN], F32)
            nc.vector.tensor_tensor(out=oh[:, :], in0=io7_sb[:, :], in1=scolr_sb[:, :], op=ALU.is_equal)
            gsel = rpool.tile([128, NB * EN], F32)
            nc.vector.tensor_tensor(out=gsel[:, :], in0=gp[:, :], in1=oh[:, :], op=ALU.mult)
            s1s = rpool.tile([128, NE], F32)
            nc.vector.tensor_reduce(out=s1s[:, :], in_=gsel.rearrange("p (t e s) -> p (t e) s", s=NS, e=E), axis=AX.X, op=ALU.add)
            score1 = rpool.tile([128, NE], F32)
            nc.vector.tensor_tensor(out=score1[:, :], in0=s1s[:, :], in1=logz[:, :], op=ALU.subtract)

            lg = rpool.tile([128, NE], F32)
            nc.scalar.activation(lg[:, :], score1[:, :], AF.Exp)     # G at scene, in (0,1)
            el = rpool.tile([128, NE], F32)
            nc.scalar.activation(el[:, :], lg[:, :], AF.Exp)         # softmax numerator
            # per-b-tile scalars ([128,1]) for the reductions' broadcasts
            ssum = rpool.tile([128, NB], F32)
            rs = rpool.tile([128, NB], F32)
            m1 = rpool.tile([128, NB], F32)
            m2 = rpool.tile([128, NB], F32)
            k1 = rpool.tile([128, NE], F32)
            k2 = rpool.tile([128, NE], F32)
            g0 = rpool.tile([128, NE], F32)
            el3 = el.rearrange("p (t e) -> p t e", e=E)
            sc3 = score1.rearrange("p (t e) -> p t e", e=E)
            q3 = q.rearrange("p (t e) -> p t e", e=E)
            nc.vector.tensor_reduce(out=ssum[:, :], in_=el3, axis=AX.X, op=ALU.add)
            nc.vector.reciprocal(rs[:, :], ssum[:, :])
            nc.vector.tensor_reduce(out=m1[:, :], in_=sc3, axis=AX.X, op=ALU.min)
            nc.vector.tensor_reduce(out=m2[:, :], in_=q3, axis=AX.X, op=ALU.min)
            for t in range(NB):
                nc.vector.tensor_scalar(
                    out=k1[:, bass.ts(t, E)], in0=score1[:, bass.ts(t, E)],
                    scalar1=m1[:, t : t + 1], scalar2=None, op0=ALU.is_equal,
                )
                nc.vector.tensor_scalar(
                    out=k2[:, bass.ts(t, E)], in0=q[:, bass.ts(t, E)],
                    scalar1=m2[:, t : t + 1], scalar2=None, op0=ALU.is_equal,
                )
                nc.vector.tensor_scalar(
                    out=g0[:, bass.ts(t, E)], in0=el[:, bass.ts(t, E)],
                    scalar1=rs[:, t : t + 1], scalar2=None, op0=ALU.mult,
                )
            kill = rpool.tile([128, NE], F32)
            nc.vector.tensor_tensor(out=kill[:, :], in0=k1[:, :], in1=k2[:, :], op=ALU.mult)
            sel = rpool.tile([128, NE], F32)
            nc.vector.tensor_scalar(
                out=sel[:, :], in0=kill[:, :], scalar1=-1.0, scalar2=1.0,
                op0=ALU.mult, op1=ALU.add,
            )
            gate_flat = gate_sb.rearrange("p t e -> p (t e)")
            nc.vector.tensor_tensor(out=gate_flat[:, :], in0=g0[:, :], in1=sel[:, :], op=ALU.mult)

        # ---- expert MLPs (bf16 matmuls, fp32 accumulation) -------------
        for e in range(E):
            # All bulk traffic rides the sync HWDGE queue in program order —
            # the per-core DMA fabric saturates at ~350GB/s regardless of
            # queue count, so ordering (not parallel queues) is what matters.
            # Two half-DMAs per weight: one trigger splits across all 16 SDMA
            # engines, and halves complete earlier than one monolithic sem.
            w1_sb = w1pool.tile([128, KT1, H1], BF16, tag="w1")
            w1_src = aps["w1"][e].rearrange("(t p) h -> p t h", p=128)
            nc.sync.dma_start(w1_sb[:, 0 : KT1 // 2, :], w1_src[:, 0 : KT1 // 2, :])
            nc.sync.dma_start(w1_sb[:, KT1 // 2 :, :], w1_src[:, KT1 // 2 :, :])
            w2_sb = w2pool.tile([128, KT2, H2], BF16, tag="w2")
            w2_src = aps["w2"][e].rearrange("(t p) o -> p t o", p=128)
            nc.sync.dma_start(w2_sb[:, 0 : KT2 // 2, :], w2_src[:, 0 : KT2 // 2, :])
            nc.sync.dma_start(w2_sb[:, KT2 // 2 :, :], w2_src[:, KT2 // 2 :, :])

            # layer 1: hT[f, b] = relu(sum_d W1[d, f] * xT[d, b] + b1[f])
            ht_sb = htpool.tile([128, KT2, BL], BF16, tag="ht")
            for m in range(MT1):
                ps = l1ps.tile([128, BL], F32, tag="ps1")
                for kt in range(KT1):
                    nc.tensor.matmul(
                        ps[:, :],
                        lhsT=w1_sb[:, kt, bass.ts(m, 128)],
                        rhs=xtb_sb[:, kt, :],
                        start=(kt == 0), stop=(kt == KT1 - 1),
                    )
                bias1 = b1_sb[:, e * MT1 + m : e * MT1 + m + 1] if has_b1 else 0.0
                nc.scalar.activation(ht_sb[:, m, :], ps[:, :], AF.Relu, bias=bias1)

            if e == 0:
                # Gate math slots in after L1(0)'s evacuations on the scalar
                # queue; it completes well before L2(0)'s first evacuation.
                routing_chain()
                rpool.release()

            # layer 2: out[b, o] = relu(sum_h hT[h, b] * W2[h, o] + b2[o])
            for mb in range(NB):
                for no in range(NO):
                    ps2 = l2ps.tile([128, 512], F32, tag="ps2")
                    for kt in range(KT2):
                        nc.tensor.matmul(
                            ps2[:, :],
                            lhsT=ht_sb[:, kt, bass.ts(mb, 128)],
                            rhs=w2_sb[:, kt, bass.ts(no, 512)],
                            start=(kt == 0),
                            stop=(kt == KT2 - 1 and not has_b2),
                        )
                    if has_b2:
                        nc.tensor.matmul(
                            ps2[:, :],
                            lhsT=ones_sb[:, :],
                            rhs=b2_sb[:, e * H2 + no * 512 : e * H2 + (no + 1) * 512],
                            start=False, stop=True,
                        )
                    gcol = gate_sb[:, mb, e : e + 1]
                    if e == 0:
                        nc.scalar.activation(
                            acc_sb[:, mb, bass.ts(no, 512)], ps2[:, :], AF.Relu, scale=gcol
                        )
                    else:
                        tmp = tmppool.tile([128, 512], F32, tag="tmp")
                        nc.scalar.activation(tmp[:, :], ps2[:, :], AF.Relu, scale=gcol)
                        nc.vector.tensor_tensor(
                            out=acc_sb[:, mb, bass.ts(no, 512)],
                            in0=acc_sb[:, mb, bass.ts(no, 512)],
                            in1=tmp[:, :], op=ALU.add,
                        )
                # Per-batch-tile output DMA so the store overlaps the
                # remaining experts' compute instead of tailing the kernel.
                if e == E - 1:
                    nc.sync.dma_start(
                        aps["out"].rearrange("(t p) o -> p t o", p=128)[:, mb, :],
                        acc_sb[:, mb, :],
                    )


def build(has_b1, has_b2):
    """Build + schedule + compile the Bass program. Returns nc."""
    nc = bacc.Bacc("TRN2", target_bir_lowering=False, debug=False)
    aps = {}
    aps["xT"] = nc.dram_tensor("xT", [D, BL], F32, kind="ExternalInput").ap()
    aps["xTb"] = nc.dram_tensor("xTb", [D, BL], BF16, kind="ExternalInput").ap()
    aps["w1"] = nc.dram_tensor("w1", [E, D, H1], BF16, kind="ExternalInput").ap()
    aps["w2"] = nc.dram_tensor("w2", [E, H1, H2], BF16, kind="ExternalInput").ap()
    if has_b1:
        aps["b1t"] = nc.dram_tensor("b1t", [128, E * MT1], F32, kind="ExternalInput").ap()
    if has_b2:
        aps["b2f"] = nc.dram_tensor("b2f", [1, E * H2], BF16, kind="ExternalInput").ap()
    aps["sflat"] = nc.dram_tensor("sflat", [D, EN], F32, kind="ExternalInput").ap()
    aps["sett"] = nc.dram_tensor("sett", [10, EN], F32, kind="ExternalInput").ap()
    aps["scol_rep"] = nc.dram_tensor("scol_rep", [128, NB * EN], F32, kind="ExternalInput").ap()
    aps["srow"] = nc.dram_tensor("srow", [1, BL], F32, kind="ExternalInput").ap()
    aps["iota7"] = nc.dram_tensor("iota7", [1, NB * EN], F32, kind="ExternalInput").ap()
    aps["iota10"] = nc.dram_tensor("iota10", [10, 1], F32, kind="ExternalInput").ap()
    aps["out"] = nc.dram_tensor("out", [BL, H2], F32, kind="ExternalOutput").ap()

    with tile.TileContext(nc) as tc:
        _emit_kernel(tc, aps, has_b1, has_b2)
    nc.compile()
    return nc


def make_in_maps(inputs):
    """Host-side layout prep + batch sharding. Returns (in_maps, has_b1, has_b2)."""
    x = np.ascontiguousarray(np.asarray(inputs["x"], dtype=np.float32))
    scene = np.asarray(inputs["scene"]).astype(np.int64)
    W1 = np.asarray(inputs["W1"], dtype=np.float32)
    b1 = np.asarray(inputs["b1"], dtype=np.float32)
    W2 = np.asarray(inputs["W2"], dtype=np.float32)
    b2 = np.asarray(inputs["b2"], dtype=np.float32)
    S = np.asarray(inputs["S"], dtype=np.float32)
    scene_emb = np.asarray(inputs["scene_emb"], dtype=np.float32)

    has_b1 = bool(np.any(b1))
    has_b2 = bool(np.any(b2))

    w1b = np.ascontiguousarray(W1.astype(NP_BF16))
    w2b = np.ascontiguousarray(W2.astype(NP_BF16))
    sflat = np.ascontiguousarray(S[:, :D, :].transpose(1, 2, 0).reshape(D, EN))
    sett = np.ascontiguousarray(
        np.einsum("rm,sme->res", scene_emb, S[:, D:, :]).reshape(scene_emb.shape[0], EN)
    )
    iota7 = np.tile(np.arange(EN, dtype=np.float32) % NS, NB).reshape(1, NB * EN)
    iota10 = np.arange(10, dtype=np.float32).reshape(10, 1)
    shared = {
        "w1": w1b, "w2": w2b, "sflat": sflat, "sett": sett,
        "iota7": iota7, "iota10": iota10,
    }
    if has_b1:
        shared["b1t"] = np.ascontiguousarray(
            b1.reshape(E, MT1, 128).transpose(2, 0, 1).reshape(128, E * MT1)
        )
    if has_b2:
        shared["b2f"] = np.ascontiguousarray(b2.astype(NP_BF16).reshape(1, E * H2))

    in_maps = []
    for c in range(N_CORES):
        xs = x[c * BL : (c + 1) * BL]
        sc = scene[c * BL : (c + 1) * BL]
        xT = np.ascontiguousarray(xs.T)
        m = dict(shared)
        m["xT"] = xT
        m["xTb"] = np.ascontiguousarray(xT.astype(NP_BF16))
        scol = sc.reshape(NB, 128).T.astype(np.float32)          # [128, NB]
        m["scol_rep"] = np.ascontiguousarray(
            np.repeat(scol[:, :, None], EN, axis=2).reshape(128, NB * EN)
        )
        m["srow"] = np.ascontiguousarray(sc.astype(np.float32).reshape(1, BL))
        in_maps.append(m)
    return in_maps, has_b1, has_b2


_NC_CACHE = {}


def get_compiled(has_b1, has_b2):
    key = (has_b1, has_b2)
    if key not in _NC_CACHE:
        _NC_CACHE[key] = build(has_b1, has_b2)
    return _NC_CACHE[key]


def run(inputs, trace=False, **kwargs):
    """Run on hardware; returns (full_output, BassKernelResults)."""
    in_maps, has_b1, has_b2 = make_in_maps(inputs)
    nc = get_compiled(has_b1, has_b2)
    res = run_bass_kernel_spmd(nc, in_maps, core_ids=list(range(N_CORES)), trace=trace, **kwargs)
    parts = [res.results[c]["out"] for c in range(N_CORES)]
    out = np.concatenate(parts, axis=0).astype(np.float32)
    full = np.ascontiguousarray(np.broadcast_to(out[None], (T, B, H2)))
    return full, res


def kernel(**inputs):
    full, _ = run(inputs, trace=False)
    return full



# revision 3
# speedup vs baseline: 1.0572x; 1.0572x over previous
"""MMoE layer kernel for 8 Trainium2 NeuronCores.

Reference math (B=4096, D=1024, H1=2048, H2=1024, E=7 experts, NS=7 scenes):
  h        = relu(einsum('bd,edh', x, W1) + b1)           # [B,E,H1]
  eo       = relu(einsum('beh,eho', h, W2) + b2)          # [B,E,H2]
  xc       = concat(x, scene_emb[scene])                  # [B, D+16]
  G        = softmax over s of einsum('bd,sde', xc, S)    # [B,E,NS] (after transpose)
  q        = mean_s log(G*7)                              # [B,E]
  score1   = logG[b, e, scene_b]
  select   = drop expert e iff e == argmin_e score1 == argmin_e q
  gate     = softmax_e(exp(score1)) * select
  out      = einsum('be,beo', gate, eo); output = stack([out, out])

Sharding: data-parallel over batch (512 rows/core), weights replicated.

Precision: expert matmuls run in bf16 except the first FP8_K rows of the
layer-1 contraction, which run as one fp8e4 DoubleRow matmul (2 k-tiles
per instruction, ~1.44x bf16 throughput). Unscaled e4m3 keeps the fp8
partial products in the same PSUM scale as the bf16 ones; measured
end-to-end metric 1.55e-2 vs the 2e-2 gate. All routing math stays fp32
so the argmin/select decisions are bit-stable.

Schedule: no PE warmup (the first real matmuls ramp the HAM clock while
the DMA queue streams expert-0 weights in consumption order); routing
matmuls run between L1(e0) and L2(e0); the last output tile is
evacuated in 128-column chunks so the kernel tail is short.

Device decomposition of the routing (no cross-partition broadcasts):
  Gpre[b, e*7+s] = x[b] @ Sflat + SE_table[scene_b]   (SE_table = scene_emb @ S[:,D:,:])
  Z = sum_s exp(Gpre); logZ = ln Z; SG = sum_s Gpre
  q      = SG/7 - logZ            (+const, argmin only)
  score1 = sum_s Gpre*onehot_s(scene) - logZ
  gate0  = softmax_e(exp(score1)) (logits in (0,1): no max-subtract needed)
  sel    = 1 - ismin(score1)*ismin(q)
  gate   = gate0 * sel
"""

import sys

if "/opt/trn_rl_repo" not in sys.path:
    sys.path.insert(0, "/opt/trn_rl_repo")

from contextlib import ExitStack

import ml_dtypes
import numpy as np

import concourse.bass as bass
import concourse.tile as tile
from concourse import bacc, mybir
from concourse.bass_utils import run_bass_kernel_spmd

F32 = mybir.dt.float32
BF16 = mybir.dt.bfloat16
FP8 = mybir.dt.float8e4
AF = mybir.ActivationFunctionType
ALU = mybir.AluOpType
AX = mybir.AxisListType
DR = mybir.MatmulPerfMode.DoubleRow

N_CORES = 8
B, D, H1, H2, E, NS, T = 4096, 1024, 2048, 1024, 7, 7, 2
BL = B // N_CORES          # 512 rows per core
NB = BL // 128             # 4 batch tiles
FP8_K = 256                # leading K rows of layer 1 in fp8 (one DoubleRow pair)
KT8 = FP8_K // 128         # 2 fp8 k-tiles
KTB = (D - FP8_K) // 128   # 6 bf16 k-tiles, layer 1
MT1 = H1 // 128            # 16 m-tiles, layer 1
NQ = 4                     # w1 column quarters (per-quarter DMA granularity)
QW = H1 // NQ              # 512 columns per quarter
KT2 = H1 // 128            # 16 k-tiles, layer 2
NO = H2 // 512             # 2  512-wide out column blocks
EN = E * NS                # 49
NP_BF16 = np.dtype(ml_dtypes.bfloat16)
NP_FP8 = np.dtype(ml_dtypes.float8_e4m3)


def _emit_kernel(tc, aps, has_b1, has_b2):
    nc = tc.nc
    ctx = ExitStack()
    with ctx:
        consts = ctx.enter_context(tc.tile_pool(name="consts", bufs=1))
        w18pool = ctx.enter_context(tc.tile_pool(name="w18", bufs=2))
        w1pool = ctx.enter_context(tc.tile_pool(name="w1", bufs=2))
        w2pool = ctx.enter_context(tc.tile_pool(name="w2", bufs=1))
        htpool = ctx.enter_context(tc.tile_pool(name="ht", bufs=1))
        tmppool = ctx.enter_context(tc.tile_pool(name="tmp", bufs=3))
        l1ps = ctx.enter_context(tc.tile_pool(name="l1ps", bufs=4, space="PSUM"))
        l2ps = ctx.enter_context(tc.tile_pool(name="l2ps", bufs=4, space="PSUM"))
        rpool = tc.alloc_tile_pool(name="routing", bufs=1)

        # ---- DMA queue in consumption order: expert-0 L1 inputs first so
        # the PE starts real work ~8.5us in (the HAM clock ramps during the
        # first 3.4us of L1(e0) instead of during dummy warmup matmuls). ---
        xq8_sb = consts.tile([128, KT8, BL], FP8)
        nc.sync.dma_start(xq8_sb[:, :, :], aps["xT8"].rearrange("(t p) b -> p t b", p=128))
        xtb_sb = consts.tile([128, KTB, BL], BF16)
        nc.sync.dma_start(xtb_sb[:, :, :], aps["xTb"].rearrange("(t p) b -> p t b", p=128))

        def dma_w1(e):
            """fp8 k-pair tile + 4 per-quarter bf16 tiles (separate tiles so
            the first expert's m-tiles start as soon as their quarter lands)."""
            w18_sb = w18pool.tile([128, KT8, H1], FP8, tag="w18")
            nc.sync.dma_start(
                w18_sb[:, :, :], aps["w18"][e].rearrange("(t p) h -> p t h", p=128)
            )
            wq = []
            for q in range(NQ):
                wq_sb = w1pool.tile([128, KTB, QW], BF16, tag=f"w1q{q}")
                nc.sync.dma_start(
                    wq_sb[:, :, :],
                    aps["w1b"][e, q].rearrange("(t p) h -> p t h", p=128),
                )
                wq.append(wq_sb)
            return w18_sb, wq

        def dma_w2(e):
            w2_sb = w2pool.tile([128, KT2, H2], BF16, tag="w2")
            w2_src = aps["w2"][e].rearrange("(t p) o -> p t o", p=128)
            nc.sync.dma_start(w2_sb[:, 0 : KT2 // 2, :], w2_src[:, 0 : KT2 // 2, :])
            nc.sync.dma_start(w2_sb[:, KT2 // 2 :, :], w2_src[:, KT2 // 2 :, :])
            return w2_sb

        w18_sb, w1q_sb = dma_w1(0)

        # ---- routing inputs follow expert-0 L1 weights; they are consumed
        # only after L1(e0) finishes. ------------------------------------
        xt_sb = rpool.tile([128, KT8 + KTB, BL], F32)
        nc.sync.dma_start(xt_sb[:, :, :], aps["xT"].rearrange("(t p) b -> p t b", p=128))
        sflat_sb = rpool.tile([128, KT8 + KTB, EN], F32)
        nc.sync.dma_start(sflat_sb[:, :, :], aps["sflat"].rearrange("(t p) j -> p t j", p=128))
        sett_sb = rpool.tile([10, EN], F32)
        nc.sync.dma_start(sett_sb[:, :], aps["sett"][:, :])
        scolr_sb = rpool.tile([128, NB * EN], F32)
        nc.sync.dma_start(scolr_sb[:, :], aps["scol_rep"][:, :])
        srow10_sb = rpool.tile([10, BL], F32)
        nc.sync.dma_start(srow10_sb[:, :], aps["srow"].to_broadcast((10, BL)))
        io7_sb = rpool.tile([128, NB * EN], F32)
        nc.sync.dma_start(io7_sb[:, :], aps["iota7"].to_broadcast((128, NB * EN)))
        io10_sb = rpool.tile([10, 1], F32)
        nc.sync.dma_start(io10_sb[:, :], aps["iota10"][:, :])

        gate_sb = consts.tile([128, NB, E], F32)
        acc_sb = consts.tile([128, NB, H2], F32)
        if has_b1:
            b1_sb = consts.tile([128, E * MT1], F32)
            nc.sync.dma_start(b1_sb[:, :], aps["b1t"][:, :])
        if has_b2:
            b2_sb = consts.tile([1, E * H2], BF16)
            nc.sync.dma_start(b2_sb[:, :], aps["b2f"][:, :])
            ones_sb = consts.tile([1, 128], BF16)
            nc.vector.memset(ones_sb[:, :], 1.0)

        w2_sb = dma_w2(0)

        def layer1(e, w18, wq):
            """hT[f, b] = relu(sum_d W1[d, f] * xT[d, b] + b1[f]); fp8
            DoubleRow covers k rows [0, FP8_K), bf16 the rest."""
            ht_sb = htpool.tile([128, KT2, BL], BF16, tag="ht")
            for m in range(MT1):
                ps = l1ps.tile([128, BL], F32, tag="ps1")
                nc.tensor.matmul(
                    ps[:, :],
                    lhsT=w18[:, 0:KT8, bass.ts(m, 128)],
                    rhs=xq8_sb[:, 0:KT8, :],
                    start=True, stop=False,
                    perf_mode=DR,
                )
                q = m // (MT1 // NQ)
                mq = m % (MT1 // NQ)
                for kt in range(KTB):
                    nc.tensor.matmul(
                        ps[:, :],
                        lhsT=wq[q][:, kt, bass.ts(mq, 128)],
                        rhs=xtb_sb[:, kt, :],
                        start=False, stop=(kt == KTB - 1),
                    )
                bias1 = b1_sb[:, e * MT1 + m : e * MT1 + m + 1] if has_b1 else 0.0
                nc.scalar.activation(ht_sb[:, m, :], ps[:, :], AF.Relu, bias=bias1)
            return ht_sb

        def routing_matmuls():
            """Gpre for all 4 b-tiles: fp32 matmuls into l2ps (free until
            L2(e0)), evacuated to gp by the routing chain."""
            psr = []
            for t in range(NB):
                psr_t = l2ps.tile([128, EN], F32, tag="ps2", name=f"psr{t}")
                psr.append(psr_t)
            # onehot over embedding rows, [10, BL]: onehot[r, b] = (scene[b] == r)
            onehot_sb = rpool.tile([10, BL], F32)
            nc.vector.tensor_scalar(
                out=onehot_sb[:, :], in0=srow10_sb[:, :],
                scalar1=io10_sb[:, 0:1], scalar2=None, op0=ALU.is_equal,
            )
            for kt in range(KT8 + KTB):
                for t in range(NB):
                    nc.tensor.matmul(
                        psr[t][:, :],
                        lhsT=xt_sb[:, kt, bass.ts(t, 128)],
                        rhs=sflat_sb[:, kt, :],
                        start=(kt == 0), stop=False,
                    )
            for t in range(NB):
                nc.tensor.matmul(
                    psr[t][:, :],
                    lhsT=onehot_sb[:, bass.ts(t, 128)],
                    rhs=sett_sb[:, :],
                    start=False, stop=True,
                )
            return psr

        def routing_chain(psr):
            """Gate computation, fused over all 4 b-tiles ([128, 4*49])."""
            NE = NB * E  # 28
            gp = rpool.tile([128, NB * EN], F32)
            for t in range(NB):
                nc.scalar.copy(gp[:, bass.ts(t, EN)], psr[t][:, :])
            gp4 = gp.rearrange("p (t e s) -> p (t e) s", s=NS, e=E)
            eex = rpool.tile([128, NB * EN], F32)
            nc.scalar.activation(eex[:, :], gp[:, :], AF.Exp)
            z = rpool.tile([128, NE], F32)
            nc.vector.tensor_reduce(out=z[:, :], in_=eex.rearrange("p (t e s) -> p (t e) s", s=NS, e=E), axis=AX.X, op=ALU.add)
            logz = rpool.tile([128, NE], F32)
            nc.scalar.activation(logz[:, :], z[:, :], AF.Ln)
            sg = rpool.tile([128, NE], F32)
            nc.vector.tensor_reduce(out=sg[:, :], in_=gp4, axis=AX.X, op=ALU.add)
            q = rpool.tile([128, NE], F32)
            nc.vector.scalar_tensor_tensor(
                out=q[:, :], in0=sg[:, :], scalar=1.0 / NS, in1=logz[:, :],
                op0=ALU.mult, op1=ALU.subtract,
            )
            oh = rpool.tile([128, NB * EN], F32)
            nc.vector.tensor_tensor(out=oh[:, :], in0=io7_sb[:, :], in1=scolr_sb[:, :], op=ALU.is_equal)
            gsel = rpool.tile([128, NB * EN], F32)
            nc.vector.tensor_tensor(out=gsel[:, :], in0=gp[:, :], in1=oh[:, :], op=ALU.mult)
            s1s = rpool.tile([128, NE], F32)
            nc.vector.tensor_reduce(out=s1s[:, :], in_=gsel.rearrange("p (t e s) -> p (t e) s", s=NS, e=E), axis=AX.X, op=ALU.add)
            score1 = rpool.tile([128, NE], F32)
            nc.vector.tensor_tensor(out=score1[:, :], in0=s1s[:, :], in1=logz[:, :], op=ALU.subtract)

            lg = rpool.tile([128, NE], F32)
            nc.scalar.activation(lg[:, :], score1[:, :], AF.Exp)     # G at scene, in (0,1)
            el = rpool.tile([128, NE], F32)
            nc.scalar.activation(el[:, :], lg[:, :], AF.Exp)         # softmax numerator
            # per-b-tile scalars ([128,1]) for the reductions' broadcasts
            ssum = rpool.tile([128, NB], F32)
            rs = rpool.tile([128, NB], F32)
            m1 = rpool.tile([128, NB], F32)
            m2 = rpool.tile([128, NB], F32)
            k1 = rpool.tile([128, NE], F32)
            k2 = rpool.tile([128, NE], F32)
            g0 = rpool.tile([128, NE], F32)
            el3 = el.rearrange("p (t e) -> p t e", e=E)
            sc3 = score1.rearrange("p (t e) -> p t e", e=E)
            q3 = q.rearrange("p (t e) -> p t e", e=E)
            nc.vector.tensor_reduce(out=ssum[:, :], in_=el3, axis=AX.X, op=ALU.add)
            nc.vector.reciprocal(rs[:, :], ssum[:, :])
            nc.vector.tensor_reduce(out=m1[:, :], in_=sc3, axis=AX.X, op=ALU.min)
            nc.vector.tensor_reduce(out=m2[:, :], in_=q3, axis=AX.X, op=ALU.min)
            for t in range(NB):
                nc.vector.tensor_scalar(
                    out=k1[:, bass.ts(t, E)], in0=score1[:, bass.ts(t, E)],
                    scalar1=m1[:, t : t + 1], scalar2=None, op0=ALU.is_equal,
                )
                nc.vector.tensor_scalar(
                    out=k2[:, bass.ts(t, E)], in0=q[:, bass.ts(t, E)],
                    scalar1=m2[:, t : t + 1], scalar2=None, op0=ALU.is_equal,
                )
                nc.vector.tensor_scalar(
                    out=g0[:, bass.ts(t, E)], in0=el[:, bass.ts(t, E)],
                    scalar1=rs[:, t : t + 1], scalar2=None, op0=ALU.mult,
                )
            kill = rpool.tile([128, NE], F32)
            nc.vector.tensor_tensor(out=kill[:, :], in0=k1[:, :], in1=k2[:, :], op=ALU.mult)
            sel = rpool.tile([128, NE], F32)
            nc.vector.tensor_scalar(
                out=sel[:, :], in0=kill[:, :], scalar1=-1.0, scalar2=1.0,
                op0=ALU.mult, op1=ALU.add,
            )
            gate_flat = gate_sb.rearrange("p t e -> p (t e)")
            nc.vector.tensor_tensor(out=gate_flat[:, :], in0=g0[:, :], in1=sel[:, :], op=ALU.mult)

        def layer2(e, ht_sb, w2_sb):
            """out[b, o] = relu(sum_h hT[h, b] * W2[h, o] + b2[o]), gated and
            accumulated into acc_sb; the last expert streams results out with
            a fine-grained final tile to keep the kernel tail short."""
            last = e == E - 1
            for mb in range(NB):
                for no in range(NO):
                    ps2 = l2ps.tile([128, 512], F32, tag="ps2")
                    for kt in range(KT2):
                        nc.tensor.matmul(
                            ps2[:, :],
                            lhsT=ht_sb[:, kt, bass.ts(mb, 128)],
                            rhs=w2_sb[:, kt, bass.ts(no, 512)],
                            start=(kt == 0),
                            stop=(kt == KT2 - 1 and not has_b2),
                        )
                    if has_b2:
                        nc.tensor.matmul(
                            ps2[:, :],
                            lhsT=ones_sb[:, :],
                            rhs=b2_sb[:, e * H2 + no * 512 : e * H2 + (no + 1) * 512],
                            start=False, stop=True,
                        )
                    gcol = gate_sb[:, mb, e : e + 1]
                    fine = last and mb == NB - 1
                    nchunk = 4 if fine else 1
                    cw = 512 // nchunk
                    for ck in range(nchunk):
                        cs = bass.ts(no * nchunk + ck, cw)
                        if e == 0:
                            nc.scalar.activation(
                                acc_sb[:, mb, cs], ps2[:, bass.ts(ck, cw)],
                                AF.Relu, scale=gcol,
                            )
                        else:
                            tmp = tmppool.tile([128, cw], F32, tag="tmp")
                            nc.scalar.activation(
                                tmp[:, :], ps2[:, bass.ts(ck, cw)], AF.Relu, scale=gcol
                            )
                            nc.vector.tensor_tensor(
                                out=acc_sb[:, mb, cs],
                                in0=acc_sb[:, mb, cs],
                                in1=tmp[:, :], op=ALU.add,
                            )
                        if fine:
                            nc.sync.dma_start(
                                aps["out"].rearrange("(t p) o -> p t o", p=128)[
                                    :, mb, no * 512 + ck * cw : no * 512 + (ck + 1) * cw
                                ],
                                acc_sb[:, mb, no * 512 + ck * cw : no * 512 + (ck + 1) * cw],
                            )
                # Per-batch-tile output DMA so the store overlaps the
                # remaining compute instead of tailing the kernel.
                if last and mb < NB - 1:
                    nc.sync.dma_start(
                        aps["out"].rearrange("(t p) o -> p t o", p=128)[:, mb, :],
                        acc_sb[:, mb, :],
                    )

        # ---- expert 0: L1, routing (PE), gate chain, L2 ----------------
        ht_sb = layer1(0, w18_sb, w1q_sb)
        psr = routing_matmuls()
        routing_chain(psr)
        rpool.release()
        # prefetch expert 1 weights behind expert 0 L2
        nw18, nw1q = dma_w1(1)
        layer2(0, ht_sb, w2_sb)

        for e in range(1, E):
            w18_sb, w1q_sb = nw18, nw1q
            w2_sb = dma_w2(e)
            ht_sb = layer1(e, w18_sb, w1q_sb)
            if e < E - 1:
                nw18, nw1q = dma_w1(e + 1)
            layer2(e, ht_sb, w2_sb)


def build(has_b1, has_b2):
    """Build + schedule + compile the Bass program. Returns nc."""
    nc = bacc.Bacc("TRN2", target_bir_lowering=False, debug=False)
    aps = {}
    aps["xT"] = nc.dram_tensor("xT", [D, BL], F32, kind="ExternalInput").ap()
    aps["xT8"] = nc.dram_tensor("xT8", [FP8_K, BL], FP8, kind="ExternalInput").ap()
    aps["xTb"] = nc.dram_tensor("xTb", [D - FP8_K, BL], BF16, kind="ExternalInput").ap()
    aps["w18"] = nc.dram_tensor("w18", [E, FP8_K, H1], FP8, kind="ExternalInput").ap()
    aps["w1b"] = nc.dram_tensor("w1b", [E, NQ, D - FP8_K, QW], BF16, kind="ExternalInput").ap()
    aps["w2"] = nc.dram_tensor("w2", [E, H1, H2], BF16, kind="ExternalInput").ap()
    if has_b1:
        aps["b1t"] = nc.dram_tensor("b1t", [128, E * MT1], F32, kind="ExternalInput").ap()
    if has_b2:
        aps["b2f"] = nc.dram_tensor("b2f", [1, E * H2], BF16, kind="ExternalInput").ap()
    aps["sflat"] = nc.dram_tensor("sflat", [D, EN], F32, kind="ExternalInput").ap()
    aps["sett"] = nc.dram_tensor("sett", [10, EN], F32, kind="ExternalInput").ap()
    aps["scol_rep"] = nc.dram_tensor("scol_rep", [128, NB * EN], F32, kind="ExternalInput").ap()
    aps["srow"] = nc.dram_tensor("srow", [1, BL], F32, kind="ExternalInput").ap()
    aps["iota7"] = nc.dram_tensor("iota7", [1, NB * EN], F32, kind="ExternalInput").ap()
    aps["iota10"] = nc.dram_tensor("iota10", [10, 1], F32, kind="ExternalInput").ap()
    aps["out"] = nc.dram_tensor("out", [BL, H2], F32, kind="ExternalOutput").ap()

    with tile.TileContext(nc) as tc:
        _emit_kernel(tc, aps, has_b1, has_b2)
    nc.compile()
    return nc


def make_in_maps(inputs):
    """Host-side layout prep + batch sharding. Returns (in_maps, has_b1, has_b2)."""
    x = np.ascontiguousarray(np.asarray(inputs["x"], dtype=np.float32))
    scene = np.asarray(inputs["scene"]).astype(np.int64)
    W1 = np.asarray(inputs["W1"], dtype=np.float32)
    b1 = np.asarray(inputs["b1"], dtype=np.float32)
    W2 = np.asarray(inputs["W2"], dtype=np.float32)
    b2 = np.asarray(inputs["b2"], dtype=np.float32)
    S = np.asarray(inputs["S"], dtype=np.float32)
    scene_emb = np.asarray(inputs["scene_emb"], dtype=np.float32)

    has_b1 = bool(np.any(b1))
    has_b2 = bool(np.any(b2))

    w18 = np.ascontiguousarray(W1[:, :FP8_K, :].astype(NP_FP8))
    w1b = np.ascontiguousarray(
        W1[:, FP8_K:, :].reshape(E, D - FP8_K, NQ, QW).transpose(0, 2, 1, 3).astype(NP_BF16)
    )
    w2b = np.ascontiguousarray(W2.astype(NP_BF16))
    sflat = np.ascontiguousarray(S[:, :D, :].transpose(1, 2, 0).reshape(D, EN))
    sett = np.ascontiguousarray(
        np.einsum("rm,sme->res", scene_emb, S[:, D:, :]).reshape(scene_emb.shape[0], EN)
    )
    iota7 = np.tile(np.arange(EN, dtype=np.float32) % NS, NB).reshape(1, NB * EN)
    iota10 = np.arange(10, dtype=np.float32).reshape(10, 1)
    shared = {
        "w18": w18, "w1b": w1b, "w2": w2b, "sflat": sflat, "sett": sett,
        "iota7": iota7, "iota10": iota10,
    }
    if has_b1:
        shared["b1t"] = np.ascontiguousarray(
            b1.reshape(E, MT1, 128).transpose(2, 0, 1).reshape(128, E * MT1)
        )
    if has_b2:
        shared["b2f"] = np.ascontiguousarray(b2.astype(NP_BF16).reshape(1, E * H2))

    in_maps = []
    for c in range(N_CORES):
        xs = x[c * BL : (c + 1) * BL]
        sc = scene[c * BL : (c + 1) * BL]
        xT = np.ascontiguousarray(xs.T)
        m = dict(shared)
        m["xT"] = xT
        m["xT8"] = np.ascontiguousarray(xT[:FP8_K].astype(NP_FP8))
        m["xTb"] = np.ascontiguousarray(xT[FP8_K:].astype(NP_BF16))
        scol = sc.reshape(NB, 128).T.astype(np.float32)          # [128, NB]
        m["scol_rep"] = np.ascontiguousarray(
            np.repeat(scol[:, :, None], EN, axis=2).reshape(128, NB * EN)
        )
        m["srow"] = np.ascontiguousarray(sc.astype(np.float32).reshape(1, BL))
        in_maps.append(m)
    return in_maps, has_b1, has_b2


_NC_CACHE = {}


def get_compiled(has_b1, has_b2):
    key = (has_b1, has_b2)
    if key not in _NC_CACHE:
        _NC_CACHE[key] = build(has_b1, has_b2)
    return _NC_CACHE[key]


def run(inputs, trace=False, **kwargs):
    """Run on hardware; returns (full_output, BassKernelResults)."""
    in_maps, has_b1, has_b2 = make_in_maps(inputs)
    nc = get_compiled(has_b1, has_b2)
    res = run_bass_kernel_spmd(nc, in_maps, core_ids=list(range(N_CORES)), trace=trace, **kwargs)
    parts = [res.results[c]["out"] for c in range(N_CORES)]
    out = np.concatenate(parts, axis=0).astype(np.float32)
    full = np.ascontiguousarray(np.broadcast_to(out[None], (T, B, H2)))
    return full, res


def kernel(**inputs):
    full, _ = run(inputs, trace=False)
    return full


# revision 8
# speedup vs baseline: 1.0588x; 1.0015x over previous
"""MMoE layer kernel for 8 Trainium2 NeuronCores.

Reference math (B=4096, D=1024, H1=2048, H2=1024, E=7 experts, NS=7 scenes):
  h        = relu(einsum('bd,edh', x, W1) + b1)           # [B,E,H1]
  eo       = relu(einsum('beh,eho', h, W2) + b2)          # [B,E,H2]
  xc       = concat(x, scene_emb[scene])                  # [B, D+16]
  G        = softmax over s of einsum('bd,sde', xc, S)    # [B,E,NS] (after transpose)
  q        = mean_s log(G*7)                              # [B,E]
  score1   = logG[b, e, scene_b]
  select   = drop expert e iff e == argmin_e score1 == argmin_e q
  gate     = softmax_e(exp(score1)) * select
  out      = einsum('be,beo', gate, eo); output = stack([out, out])

Sharding: data-parallel over batch (512 rows/core), weights replicated.

Precision: expert matmuls run in bf16 except the first FP8_K rows of the
layer-1 contraction, which run as one fp8e4 DoubleRow matmul (2 k-tiles
per instruction, ~1.4x bf16 throughput). Unscaled e4m3 keeps the fp8
partial products in the same PSUM scale as the bf16 ones; measured
end-to-end metric 1.55e-2 vs the 2e-2 gate.

Routing runs reversed ([49, B] out = S^T x) as bf16 hi/lo 3-term matmuls
(xh*Sh + xl*Sh + xh*Sl, max logit err ~1.3e-5, 40-100x below the
smallest argmin gap so select stays bit-stable), then PE-transposes back
to [128, 49]-per-b-tile for the fp32 gate chain. This replaces 4x-cost
fp32 matmuls and runs FIRST, overlapping the expert-weight DMA prologue
and absorbing the HAM cold-clock ramp with cheap instructions.
"""

import sys

if "/opt/trn_rl_repo" not in sys.path:
    sys.path.insert(0, "/opt/trn_rl_repo")

from contextlib import ExitStack

import ml_dtypes
import numpy as np

import concourse.bass as bass
import concourse.tile as tile
from concourse import bacc, mybir
from concourse.bass_utils import run_bass_kernel_spmd

F32 = mybir.dt.float32
BF16 = mybir.dt.bfloat16
FP8 = mybir.dt.float8e4
AF = mybir.ActivationFunctionType
ALU = mybir.AluOpType
AX = mybir.AxisListType
DR = mybir.MatmulPerfMode.DoubleRow

N_CORES = 8
B, D, H1, H2, E, NS, T = 4096, 1024, 2048, 1024, 7, 7, 2
BL = B // N_CORES          # 512 rows per core
NB = BL // 128             # 4 batch tiles
FP8_K = 256                # leading K rows of layer 1 in fp8 (one DoubleRow pair)
KT8 = FP8_K // 128         # 2 fp8 k-tiles
KTB = (D - FP8_K) // 128   # 6 bf16 k-tiles, layer 1
KT = D // 128              # 8 k-tiles of x (routing)
MT1 = H1 // 128            # 16 m-tiles, layer 1
NQ = 4                     # w1 column quarters for expert 0 (DMA granularity)
QW = H1 // NQ              # 512 columns per quarter
KT2 = H1 // 128            # 16 k-tiles, layer 2
NO = H2 // 512             # 2  512-wide out column blocks
EN = E * NS                # 49
WARM = 12                  # HAM clock-ramp matmuls while the DMA queue boots
NP_BF16 = np.dtype(ml_dtypes.bfloat16)
NP_FP8 = np.dtype(ml_dtypes.float8_e4m3)


def _emit_kernel(tc, aps, has_b1, has_b2):
    nc = tc.nc
    ctx = ExitStack()
    with ctx:
        consts = ctx.enter_context(tc.tile_pool(name="consts", bufs=1))
        w18pool = ctx.enter_context(tc.tile_pool(name="w18", bufs=2))
        w1pool = ctx.enter_context(tc.tile_pool(name="w1", bufs=2))
        w2pool = ctx.enter_context(tc.tile_pool(name="w2", bufs=1))
        htpool = ctx.enter_context(tc.tile_pool(name="ht", bufs=1))
        tmppool = ctx.enter_context(tc.tile_pool(name="tmp", bufs=3))
        l1ps = ctx.enter_context(tc.tile_pool(name="l1ps", bufs=4, space="PSUM"))
        l2ps = ctx.enter_context(tc.tile_pool(name="l2ps", bufs=4, space="PSUM"))
        rpool = tc.alloc_tile_pool(name="routing", bufs=1)

        # ---- DMA queue in consumption order: routing inputs, then expert-0
        # L1 inputs, then everything else. -------------------------------
        sfh_sb = rpool.tile([128, KT, EN], BF16)
        nc.sync.dma_start(sfh_sb[:, :, :], aps["sfh"].rearrange("(t p) j -> p t j", p=128))
        sfl_sb = rpool.tile([128, KT, EN], BF16)
        nc.sync.dma_start(sfl_sb[:, :, :], aps["sfl"].rearrange("(t p) j -> p t j", p=128))
        setth_sb = rpool.tile([10, EN], BF16)
        nc.sync.dma_start(setth_sb[:, :], aps["setth"][:, :])
        settl_sb = rpool.tile([10, EN], BF16)
        nc.sync.dma_start(settl_sb[:, :], aps["settl"][:, :])
        srow10_sb = rpool.tile([10, BL], F32)
        nc.sync.dma_start(srow10_sb[:, :], aps["srow"].to_broadcast((10, BL)))
        io10_sb = rpool.tile([10, 1], F32)
        nc.sync.dma_start(io10_sb[:, :], aps["iota10"][:, :])
        ident_sb = rpool.tile([EN, EN], F32)
        nc.sync.dma_start(ident_sb[:, :], aps["ident"][:, :])
        xfull_sb = consts.tile([128, KT, BL], BF16)
        nc.sync.dma_start(xfull_sb[:, :, :], aps["xh"].rearrange("(t p) b -> p t b", p=128))
        xl_sb = rpool.tile([128, KT, BL], BF16)
        nc.sync.dma_start(xl_sb[:, :, :], aps["xl"].rearrange("(t p) b -> p t b", p=128))
        xq8_sb = consts.tile([128, KT8, BL], FP8)
        nc.sync.dma_start(xq8_sb[:, :, :], aps["xT8"].rearrange("(t p) b -> p t b", p=128))

        def dma_w1(e):
            """fp8 k-pair tile + bf16 tail; expert 0 gets per-quarter tiles so
            its first m-tiles start as soon as their quarter lands, later
            experts one tile (fewer first-read semaphore waits on the PE)."""
            w18_sb = w18pool.tile([128, KT8, H1], FP8, tag="w18")
            nc.sync.dma_start(
                w18_sb[:, :, :], aps["w18"][e].rearrange("(t p) h -> p t h", p=128)
            )
            wf_sb = w1pool.tile([128, KTB, H1], BF16, tag="w1f")
            wsrc = aps["w1b"][e].rearrange("(t p) h -> p t h", p=128)
            nc.sync.dma_start(wf_sb[:, :, 0 : H1 // 2], wsrc[:, :, 0 : H1 // 2])
            nc.sync.dma_start(wf_sb[:, :, H1 // 2 :], wsrc[:, :, H1 // 2 :])
            lhs = lambda m, kt: wf_sb[:, kt, bass.ts(m, 128)]
            return w18_sb, lhs

        def dma_w2(e):
            w2_sb = w2pool.tile([128, KT2, H2], BF16, tag="w2")
            w2_src = aps["w2"][e].rearrange("(t p) o -> p t o", p=128)
            nc.sync.dma_start(w2_sb[:, 0 : KT2 // 2, :], w2_src[:, 0 : KT2 // 2, :])
            nc.sync.dma_start(w2_sb[:, KT2 // 2 :, :], w2_src[:, KT2 // 2 :, :])
            return w2_sb

        scolr_sb = rpool.tile([128, NB * EN], F32)
        nc.sync.dma_start(scolr_sb[:, :], aps["scol_rep"][:, :])
        io7_sb = rpool.tile([128, NB * EN], F32)
        nc.sync.dma_start(io7_sb[:, :], aps["iota7"].to_broadcast((128, NB * EN)))

        gate_sb = consts.tile([128, NB, E], F32)
        acc_sb = consts.tile([128, NB, H2], F32)
        if has_b1:
            b1_sb = consts.tile([128, E * MT1], F32)
            nc.sync.dma_start(b1_sb[:, :], aps["b1t"][:, :])
        if has_b2:
            b2_sb = consts.tile([1, E * H2], BF16)
            nc.sync.dma_start(b2_sb[:, :], aps["b2f"][:, :])
            ones_sb = consts.tile([1, 128], BF16)
            nc.vector.memset(ones_sb[:, :], 1.0)

        w18_sb, w1lhs = dma_w1(0)
        w2_sb = dma_w2(0)

        # ---- PE warm-up while the DMA engines boot (~13us before the
        # routing inputs land); ramps the HAM clock gate to 8/8. ----------
        warm_sb = rpool.tile([128, 640], BF16)
        nc.vector.memset(warm_sb[:, :], 0.0)
        warm_ps = l1ps.tile([128, 512], F32, tag="ps1", name="warm_ps")
        for _ in range(WARM):
            nc.tensor.matmul(
                warm_ps[:, :], lhsT=warm_sb[:, 0:128], rhs=warm_sb[:, 0:512],
                start=True, stop=True,
            )

        # onehot over embedding rows, bf16 [10, BL]: onehot[r, b] = (scene[b] == r)
        oh16_sb = rpool.tile([10, BL], BF16)
        nc.vector.tensor_scalar(
            out=oh16_sb[:, :], in0=srow10_sb[:, :],
            scalar1=io10_sb[:, 0:1], scalar2=None, op0=ALU.is_equal,
        )

        # ---- routing, reversed layout: psg[j, b] = sum_d sflat[d, j] x[d, b]
        # + sett[scene_b, j], computed as bf16 hi/lo three-term sums. ------
        psg = l2ps.tile([128, BL], F32, tag="ps2", name="psg")
        for kt in range(KT):
            nc.tensor.matmul(
                psg[0:EN, :], lhsT=sfh_sb[:, kt, :], rhs=xfull_sb[:, kt, :],
                start=(kt == 0), stop=False,
            )
            nc.tensor.matmul(
                psg[0:EN, :], lhsT=sfl_sb[:, kt, :], rhs=xfull_sb[:, kt, :],
                start=False, stop=False,
            )
            nc.tensor.matmul(
                psg[0:EN, :], lhsT=sfh_sb[:, kt, :], rhs=xl_sb[:, kt, :],
                start=False, stop=False,
            )
        nc.tensor.matmul(
            psg[0:EN, :], lhsT=setth_sb[:, :], rhs=oh16_sb[:, :],
            start=False, stop=False,
        )
        nc.tensor.matmul(
            psg[0:EN, :], lhsT=settl_sb[:, :], rhs=oh16_sb[:, :],
            start=False, stop=True,
        )
        gsb = rpool.tile([EN, BL], F32)
        nc.scalar.copy(gsb[:, :], psg[0:EN, :])

        def layer1(e, w18, w1lhs):
            """hT[f, b] = relu(sum_d W1[d, f] * xT[d, b] + b1[f]); fp8
            DoubleRow covers k rows [0, FP8_K), bf16 the rest."""
            ht_sb = htpool.tile([128, KT2, BL], BF16, tag="ht")
            for m in range(MT1):
                ps = l1ps.tile([128, BL], F32, tag="ps1")
                nc.tensor.matmul(
                    ps[:, :],
                    lhsT=w18[:, 0:KT8, bass.ts(m, 128)],
                    rhs=xq8_sb[:, 0:KT8, :],
                    start=True, stop=False,
                    perf_mode=DR,
                )
                for kt in range(KTB):
                    nc.tensor.matmul(
                        ps[:, :],
                        lhsT=w1lhs(m, kt),
                        rhs=xfull_sb[:, KT8 + kt, :],
                        start=False, stop=(kt == KTB - 1),
                    )
                bias1 = b1_sb[:, e * MT1 + m : e * MT1 + m + 1] if has_b1 else 0.0
                nc.scalar.activation(ht_sb[:, m, :], ps[:, :], AF.Relu, bias=bias1)
            return ht_sb

        def transposes():
            """[49, BL] routing result -> four [128, 49] b-tiles via PE."""
            psr = []
            for t in range(NB):
                psr_t = l1ps.tile([128, EN], F32, tag="ps1", name=f"psr{t}")
                nc.tensor.matmul(
                    psr_t[:, :], lhsT=gsb[:, bass.ts(t, 128)], rhs=ident_sb[:, :],
                    is_transpose=True,
                )
                psr.append(psr_t)
            return psr

        def routing_chain(psr):
            """Gate computation, fused over all 4 b-tiles ([128, 4*49])."""
            NE = NB * E  # 28
            gp = rpool.tile([128, NB * EN], F32)
            for t in range(NB):
                nc.scalar.copy(gp[:, bass.ts(t, EN)], psr[t][:, :])
            gp4 = gp.rearrange("p (t e s) -> p (t e) s", s=NS, e=E)
            eex = rpool.tile([128, NB * EN], F32)
            nc.scalar.activation(eex[:, :], gp[:, :], AF.Exp)
            z = rpool.tile([128, NE], F32)
            nc.vector.tensor_reduce(out=z[:, :], in_=eex.rearrange("p (t e s) -> p (t e) s", s=NS, e=E), axis=AX.X, op=ALU.add)
            logz = rpool.tile([128, NE], F32)
            nc.scalar.activation(logz[:, :], z[:, :], AF.Ln)
            sg = rpool.tile([128, NE], F32)
            nc.vector.tensor_reduce(out=sg[:, :], in_=gp4, axis=AX.X, op=ALU.add)
            q = rpool.tile([128, NE], F32)
            nc.vector.scalar_tensor_tensor(
                out=q[:, :], in0=sg[:, :], scalar=1.0 / NS, in1=logz[:, :],
                op0=ALU.mult, op1=ALU.subtract,
            )
            oh = rpool.tile([128, NB * EN], F32)
            nc.vector.tensor_tensor(out=oh[:, :], in0=io7_sb[:, :], in1=scolr_sb[:, :], op=ALU.is_equal)
            gsel = rpool.tile([128, NB * EN], F32)
            nc.vector.tensor_tensor(out=gsel[:, :], in0=gp[:, :], in1=oh[:, :], op=ALU.mult)
            s1s = rpool.tile([128, NE], F32)
            nc.vector.tensor_reduce(out=s1s[:, :], in_=gsel.rearrange("p (t e s) -> p (t e) s", s=NS, e=E), axis=AX.X, op=ALU.add)
            score1 = rpool.tile([128, NE], F32)
            nc.vector.tensor_tensor(out=score1[:, :], in0=s1s[:, :], in1=logz[:, :], op=ALU.subtract)

            lg = rpool.tile([128, NE], F32)
            nc.scalar.activation(lg[:, :], score1[:, :], AF.Exp)     # G at scene, in (0,1)
            el = rpool.tile([128, NE], F32)
            nc.scalar.activation(el[:, :], lg[:, :], AF.Exp)         # softmax numerator
            # per-b-tile scalars ([128,1]) for the reductions' broadcasts
            ssum = rpool.tile([128, NB], F32)
            rs = rpool.tile([128, NB], F32)
            m1 = rpool.tile([128, NB], F32)
            m2 = rpool.tile([128, NB], F32)
            k1 = rpool.tile([128, NE], F32)
            k2 = rpool.tile([128, NE], F32)
            g0 = rpool.tile([128, NE], F32)
            el3 = el.rearrange("p (t e) -> p t e", e=E)
            sc3 = score1.rearrange("p (t e) -> p t e", e=E)
            q3 = q.rearrange("p (t e) -> p t e", e=E)
            nc.vector.tensor_reduce(out=ssum[:, :], in_=el3, axis=AX.X, op=ALU.add)
            nc.vector.reciprocal(rs[:, :], ssum[:, :])
            nc.vector.tensor_reduce(out=m1[:, :], in_=sc3, axis=AX.X, op=ALU.min)
            nc.vector.tensor_reduce(out=m2[:, :], in_=q3, axis=AX.X, op=ALU.min)
            for t in range(NB):
                nc.vector.tensor_scalar(
                    out=k1[:, bass.ts(t, E)], in0=score1[:, bass.ts(t, E)],
                    scalar1=m1[:, t : t + 1], scalar2=None, op0=ALU.is_equal,
                )
                nc.vector.tensor_scalar(
                    out=k2[:, bass.ts(t, E)], in0=q[:, bass.ts(t, E)],
                    scalar1=m2[:, t : t + 1], scalar2=None, op0=ALU.is_equal,
                )
                nc.vector.tensor_scalar(
                    out=g0[:, bass.ts(t, E)], in0=el[:, bass.ts(t, E)],
                    scalar1=rs[:, t : t + 1], scalar2=None, op0=ALU.mult,
                )
            kill = rpool.tile([128, NE], F32)
            nc.vector.tensor_tensor(out=kill[:, :], in0=k1[:, :], in1=k2[:, :], op=ALU.mult)
            sel = rpool.tile([128, NE], F32)
            nc.vector.tensor_scalar(
                out=sel[:, :], in0=kill[:, :], scalar1=-1.0, scalar2=1.0,
                op0=ALU.mult, op1=ALU.add,
            )
            gate_flat = gate_sb.rearrange("p t e -> p (t e)")
            nc.vector.tensor_tensor(out=gate_flat[:, :], in0=g0[:, :], in1=sel[:, :], op=ALU.mult)

        def layer2(e, ht_sb, w2_sb):
            """out[b, o] = relu(sum_h hT[h, b] * W2[h, o] + b2[o]), gated and
            accumulated into acc_sb; last expert streams the result out."""
            last = e == E - 1
            for mb in range(NB):
                for no in range(NO):
                    ps2 = l2ps.tile([128, 512], F32, tag="ps2")
                    for kt in range(KT2):
                        nc.tensor.matmul(
                            ps2[:, :],
                            lhsT=ht_sb[:, kt, bass.ts(mb, 128)],
                            rhs=w2_sb[:, kt, bass.ts(no, 512)],
                            start=(kt == 0),
                            stop=(kt == KT2 - 1 and not has_b2),
                        )
                    if has_b2:
                        nc.tensor.matmul(
                            ps2[:, :],
                            lhsT=ones_sb[:, :],
                            rhs=b2_sb[:, e * H2 + no * 512 : e * H2 + (no + 1) * 512],
                            start=False, stop=True,
                        )
                    gcol = gate_sb[:, mb, e : e + 1]
                    if e == 0:
                        nc.scalar.activation(
                            acc_sb[:, mb, bass.ts(no, 512)], ps2[:, :], AF.Relu, scale=gcol
                        )
                    else:
                        tmp = tmppool.tile([128, 512], F32, tag="tmp")
                        nc.scalar.activation(tmp[:, :], ps2[:, :], AF.Relu, scale=gcol)
                        nc.vector.tensor_tensor(
                            out=acc_sb[:, mb, bass.ts(no, 512)],
                            in0=acc_sb[:, mb, bass.ts(no, 512)],
                            in1=tmp[:, :], op=ALU.add,
                        )
                    # Last batch tile of the last expert: store each 512-col
                    # half as soon as its add lands, shortening the tail.
                    if last and mb == NB - 1:
                        nc.sync.dma_start(
                            aps["out"].rearrange("(t p) o -> p t o", p=128)[
                                :, mb, bass.ts(no, 512)
                            ],
                            acc_sb[:, mb, bass.ts(no, 512)],
                        )
                if last and mb < NB - 1:
                    nc.sync.dma_start(
                        aps["out"].rearrange("(t p) o -> p t o", p=128)[:, mb, :],
                        acc_sb[:, mb, :],
                    )

        # ---- expert 0: L1 (while routing result waits), transpose+gate, L2
        ht_sb = layer1(0, w18_sb, w1lhs)
        psr = transposes()
        routing_chain(psr)
        rpool.release()
        nw18, nw1lhs = dma_w1(1)
        layer2(0, ht_sb, w2_sb)

        for e in range(1, E):
            w18_sb, w1lhs = nw18, nw1lhs
            w2_sb = dma_w2(e)
            ht_sb = layer1(e, w18_sb, w1lhs)
            if e < E - 1:
                nw18, nw1lhs = dma_w1(e + 1)
            layer2(e, ht_sb, w2_sb)


def build(has_b1, has_b2):
    """Build + schedule + compile the Bass program. Returns nc."""
    nc = bacc.Bacc("TRN2", target_bir_lowering=False, debug=False)
    aps = {}
    aps["xh"] = nc.dram_tensor("xh", [D, BL], BF16, kind="ExternalInput").ap()
    aps["xl"] = nc.dram_tensor("xl", [D, BL], BF16, kind="ExternalInput").ap()
    aps["xT8"] = nc.dram_tensor("xT8", [FP8_K, BL], FP8, kind="ExternalInput").ap()
    aps["w18"] = nc.dram_tensor("w18", [E, FP8_K, H1], FP8, kind="ExternalInput").ap()
    aps["w1b"] = nc.dram_tensor("w1b", [E, D - FP8_K, H1], BF16, kind="ExternalInput").ap()
    aps["w2"] = nc.dram_tensor("w2", [E, H1, H2], BF16, kind="ExternalInput").ap()
    if has_b1:
        aps["b1t"] = nc.dram_tensor("b1t", [128, E * MT1], F32, kind="ExternalInput").ap()
    if has_b2:
        aps["b2f"] = nc.dram_tensor("b2f", [1, E * H2], BF16, kind="ExternalInput").ap()
    aps["sfh"] = nc.dram_tensor("sfh", [D, EN], BF16, kind="ExternalInput").ap()
    aps["sfl"] = nc.dram_tensor("sfl", [D, EN], BF16, kind="ExternalInput").ap()
    aps["setth"] = nc.dram_tensor("setth", [10, EN], BF16, kind="ExternalInput").ap()
    aps["settl"] = nc.dram_tensor("settl", [10, EN], BF16, kind="ExternalInput").ap()
    aps["ident"] = nc.dram_tensor("ident", [EN, EN], F32, kind="ExternalInput").ap()
    aps["scol_rep"] = nc.dram_tensor("scol_rep", [128, NB * EN], F32, kind="ExternalInput").ap()
    aps["srow"] = nc.dram_tensor("srow", [1, BL], F32, kind="ExternalInput").ap()
    aps["iota7"] = nc.dram_tensor("iota7", [1, NB * EN], F32, kind="ExternalInput").ap()
    aps["iota10"] = nc.dram_tensor("iota10", [10, 1], F32, kind="ExternalInput").ap()
    aps["out"] = nc.dram_tensor("out", [BL, H2], F32, kind="ExternalOutput").ap()

    with tile.TileContext(nc) as tc:
        _emit_kernel(tc, aps, has_b1, has_b2)
    nc.compile()
    return nc


def make_in_maps(inputs):
    """Host-side layout prep + batch sharding. Returns (in_maps, has_b1, has_b2)."""
    x = np.ascontiguousarray(np.asarray(inputs["x"], dtype=np.float32))
    scene = np.asarray(inputs["scene"]).astype(np.int64)
    W1 = np.asarray(inputs["W1"], dtype=np.float32)
    b1 = np.asarray(inputs["b1"], dtype=np.float32)
    W2 = np.asarray(inputs["W2"], dtype=np.float32)
    b2 = np.asarray(inputs["b2"], dtype=np.float32)
    S = np.asarray(inputs["S"], dtype=np.float32)
    scene_emb = np.asarray(inputs["scene_emb"], dtype=np.float32)

    has_b1 = bool(np.any(b1))
    has_b2 = bool(np.any(b2))

    def hilo(a):
        h = a.astype(NP_BF16)
        l = (a - h.astype(np.float32)).astype(NP_BF16)
        return np.ascontiguousarray(h), np.ascontiguousarray(l)

    w18 = np.ascontiguousarray(W1[:, :FP8_K, :].astype(NP_FP8))
    w1b = np.ascontiguousarray(W1[:, FP8_K:, :].astype(NP_BF16))
    w2b = np.ascontiguousarray(W2.astype(NP_BF16))
    sflat = np.ascontiguousarray(S[:, :D, :].transpose(1, 2, 0).reshape(D, EN))
    sfh, sfl = hilo(sflat)
    sett = np.ascontiguousarray(
        np.einsum("rm,sme->res", scene_emb, S[:, D:, :]).reshape(scene_emb.shape[0], EN)
    )
    setth, settl = hilo(sett)
    iota7 = np.tile(np.arange(EN, dtype=np.float32) % NS, NB).reshape(1, NB * EN)
    iota10 = np.arange(10, dtype=np.float32).reshape(10, 1)
    ident = np.eye(EN, dtype=np.float32)
    shared = {
        "w18": w18, "w1b": w1b, "w2": w2b, "sfh": sfh, "sfl": sfl,
        "setth": setth, "settl": settl, "ident": ident,
        "iota7": iota7, "iota10": iota10,
    }
    if has_b1:
        shared["b1t"] = np.ascontiguousarray(
            b1.reshape(E, MT1, 128).transpose(2, 0, 1).reshape(128, E * MT1)
        )
    if has_b2:
        shared["b2f"] = np.ascontiguousarray(b2.astype(NP_BF16).reshape(1, E * H2))

    in_maps = []
    for c in range(N_CORES):
        xs = x[c * BL : (c + 1) * BL]
        sc = scene[c * BL : (c + 1) * BL]
        xT = np.ascontiguousarray(xs.T)
        m = dict(shared)
        m["xh"], m["xl"] = hilo(xT)
        m["xT8"] = np.ascontiguousarray(xT[:FP8_K].astype(NP_FP8))
        scol = sc.reshape(NB, 128).T.astype(np.float32)          # [128, NB]
        m["scol_rep"] = np.ascontiguousarray(
            np.repeat(scol[:, :, None], EN, axis=2).reshape(128, NB * EN)
        )
        m["srow"] = np.ascontiguousarray(sc.astype(np.float32).reshape(1, BL))
        in_maps.append(m)
    return in_maps, has_b1, has_b2


_NC_CACHE = {}


def get_compiled(has_b1, has_b2):
    key = (has_b1, has_b2)
    if key not in _NC_CACHE:
        _NC_CACHE[key] = build(has_b1, has_b2)
    return _NC_CACHE[key]


def run(inputs, trace=False, **kwargs):
    """Run on hardware; returns (full_output, BassKernelResults)."""
    in_maps, has_b1, has_b2 = make_in_maps(inputs)
    nc = get_compiled(has_b1, has_b2)
    res = run_bass_kernel_spmd(nc, in_maps, core_ids=list(range(N_CORES)), trace=trace, **kwargs)
    parts = [res.results[c]["out"] for c in range(N_CORES)]
    out = np.concatenate(parts, axis=0).astype(np.float32)
    full = np.ascontiguousarray(np.broadcast_to(out[None], (T, B, H2)))
    return full, res


def kernel(**inputs):
    full, _ = run(inputs, trace=False)
    return full


# revision 9
# speedup vs baseline: 1.0750x; 1.0154x over previous
"""MMoE layer kernel for 8 Trainium2 NeuronCores.

Reference math (B=4096, D=1024, H1=2048, H2=1024, E=7 experts, NS=7 scenes):
  h        = relu(einsum('bd,edh', x, W1) + b1)           # [B,E,H1]
  eo       = relu(einsum('beh,eho', h, W2) + b2)          # [B,E,H2]
  xc       = concat(x, scene_emb[scene])                  # [B, D+16]
  G        = softmax over s of einsum('bd,sde', xc, S)    # [B,E,NS] (after transpose)
  q        = mean_s log(G*7)                              # [B,E]
  score1   = logG[b, e, scene_b]
  select   = drop expert e iff e == argmin_e score1 == argmin_e q
  gate     = softmax_e(exp(score1)) * select
  out      = einsum('be,beo', gate, eo); output = stack([out, out])

Sharding: data-parallel over batch (512 rows/core), weights replicated.

Precision: expert matmuls run in bf16 except the first FP8_K rows of the
layer-1 contraction, which run as one fp8e4 DoubleRow matmul (2 k-tiles
per instruction, ~1.4x bf16 throughput). Unscaled e4m3 keeps the fp8
partial products in the same PSUM scale as the bf16 ones; measured
end-to-end metric 1.55e-2 vs the 2e-2 gate.

Routing runs reversed ([49, B] out = S^T x) as bf16 hi/lo 3-term matmuls
(xh*Sh + xl*Sh + xh*Sl, max logit err ~1.3e-5, 40-100x below the
smallest argmin gap so select stays bit-stable), then PE-transposes back
to [128, 49]-per-b-tile for the fp32 gate chain. This replaces 4x-cost
fp32 matmuls and runs FIRST, overlapping the expert-weight DMA prologue
and absorbing the HAM cold-clock ramp with cheap instructions.
"""

import sys

if "/opt/trn_rl_repo" not in sys.path:
    sys.path.insert(0, "/opt/trn_rl_repo")

from contextlib import ExitStack

import ml_dtypes
import numpy as np

import concourse.bass as bass
import concourse.tile as tile
from concourse import bacc, mybir
from concourse.bass_utils import run_bass_kernel_spmd

F32 = mybir.dt.float32
BF16 = mybir.dt.bfloat16
FP8 = mybir.dt.float8e4
AF = mybir.ActivationFunctionType
ALU = mybir.AluOpType
AX = mybir.AxisListType
DR = mybir.MatmulPerfMode.DoubleRow

N_CORES = 8
B, D, H1, H2, E, NS, T = 4096, 1024, 2048, 1024, 7, 7, 2
BL = B // N_CORES          # 512 rows per core
NB = BL // 128             # 4 batch tiles
FP8_K = 256                # leading K rows of layer 1 in fp8 (one DoubleRow pair)
KT8 = FP8_K // 128         # 2 fp8 k-tiles
KTB = (D - FP8_K) // 128   # 6 bf16 k-tiles, layer 1
KT = D // 128              # 8 k-tiles of x (routing)
MT1 = H1 // 128            # 16 m-tiles, layer 1
NQ = 4                     # w1 column quarters for expert 0 (DMA granularity)
QW = H1 // NQ              # 512 columns per quarter
KT2 = H1 // 128            # 16 k-tiles, layer 2
NO = H2 // 512             # 2  512-wide out column blocks
EN = E * NS                # 49
WARM = 13                  # HAM clock-ramp matmuls while the DMA queue boots
NP_BF16 = np.dtype(ml_dtypes.bfloat16)
NP_FP8 = np.dtype(ml_dtypes.float8_e4m3)


def _emit_kernel(tc, aps, has_b1, has_b2):
    nc = tc.nc
    ctx = ExitStack()
    with ctx:
        consts = ctx.enter_context(tc.tile_pool(name="consts", bufs=1))
        w18pool = ctx.enter_context(tc.tile_pool(name="w18", bufs=2))
        w1pool = ctx.enter_context(tc.tile_pool(name="w1", bufs=2))
        w2pool = ctx.enter_context(tc.tile_pool(name="w2", bufs=1))
        htpool = ctx.enter_context(tc.tile_pool(name="ht", bufs=1))
        tmppool = ctx.enter_context(tc.tile_pool(name="tmp", bufs=3))
        l1ps = ctx.enter_context(tc.tile_pool(name="l1ps", bufs=4, space="PSUM"))
        l2ps = ctx.enter_context(tc.tile_pool(name="l2ps", bufs=4, space="PSUM"))
        rpool = tc.alloc_tile_pool(name="routing", bufs=1)

        # ---- DMA queue in consumption order: routing inputs, then expert-0
        # L1 inputs, then everything else. -------------------------------
        sfh_sb = rpool.tile([128, KT, EN], BF16)
        nc.sync.dma_start(sfh_sb[:, :, :], aps["sfh"][:, :])
        sfl_sb = rpool.tile([128, KT, EN], BF16)
        nc.sync.dma_start(sfl_sb[:, :, :], aps["sfl"][:, :])
        xfull_sb = consts.tile([128, KT, BL], BF16)
        nc.sync.dma_start(xfull_sb[:, :, :], aps["xh"][:, :])
        xl_sb = rpool.tile([128, KT, BL], BF16)
        nc.sync.dma_start(xl_sb[:, :, :], aps["xl"][:, :])
        srow10_sb = rpool.tile([10, BL], F32)
        nc.sync.dma_start(srow10_sb[:, :], aps["srow"].to_broadcast((10, BL)))
        io10_sb = rpool.tile([10, 1], F32)
        nc.sync.dma_start(io10_sb[:, :], aps["iota10"][:, :])
        setth_sb = rpool.tile([10, EN], BF16)
        nc.sync.dma_start(setth_sb[:, :], aps["setth"][:, :])
        settl_sb = rpool.tile([10, EN], BF16)
        nc.sync.dma_start(settl_sb[:, :], aps["settl"][:, :])
        xq8_sb = consts.tile([128, KT8, BL], FP8)
        nc.sync.dma_start(xq8_sb[:, :, :], aps["xT8"][:, :])

        def dma_w1(e):
            """fp8 k-pair tile + bf16 tail; expert 0 gets per-quarter tiles so
            its first m-tiles start as soon as their quarter lands, later
            experts one tile (fewer first-read semaphore waits on the PE)."""
            w18_sb = w18pool.tile([128, KT8, H1], FP8, tag="w18")
            nc.sync.dma_start(w18_sb[:, :, :], aps["w18"][e])
            wf_sb = w1pool.tile([128, KTB, H1], BF16, tag="w1f")
            wsrc = aps["w1b"][e].rearrange("p (t h) -> p t h", h=H1)
            nc.sync.dma_start(wf_sb[:, :, 0 : H1 // 2], wsrc[:, :, 0 : H1 // 2])
            nc.sync.dma_start(wf_sb[:, :, H1 // 2 :], wsrc[:, :, H1 // 2 :])
            lhs = lambda m, kt: wf_sb[:, kt, bass.ts(m, 128)]
            return w18_sb, lhs

        def dma_w2(e):
            w2_sb = w2pool.tile([128, KT2, H2], BF16, tag="w2")
            w2_src = aps["w2"][e].rearrange("p (t o) -> p t o", o=H2)
            nc.sync.dma_start(w2_sb[:, 0 : KT2 // 2, :], w2_src[:, 0 : KT2 // 2, :])
            nc.sync.dma_start(w2_sb[:, KT2 // 2 :, :], w2_src[:, KT2 // 2 :, :])
            return w2_sb

        gate_sb = consts.tile([128, NB, E], F32)
        acc_sb = consts.tile([128, NB, H2], F32)
        if has_b1:
            b1_sb = consts.tile([128, E * MT1], F32)
            nc.sync.dma_start(b1_sb[:, :], aps["b1t"][:, :])
        if has_b2:
            b2_sb = consts.tile([1, E * H2], BF16)
            nc.sync.dma_start(b2_sb[:, :], aps["b2f"][:, :])
            ones_sb = consts.tile([1, 128], BF16)
            nc.vector.memset(ones_sb[:, :], 1.0)

        w18_sb, w1lhs = dma_w1(0)
        ident_sb = rpool.tile([EN, EN], F32)
        nc.sync.dma_start(ident_sb[:, :], aps["ident"][:, :])
        scolr_sb = rpool.tile([128, NB * EN], F32)
        nc.sync.dma_start(scolr_sb[:, :], aps["scol_rep"][:, :])
        io7_sb = rpool.tile([128, NB * EN], F32)
        nc.sync.dma_start(io7_sb[:, :], aps["iota7"].to_broadcast((128, NB * EN)))
        w2_sb = dma_w2(0)

        # ---- PE warm-up while the DMA engines boot (~13us before the
        # routing inputs land); ramps the HAM clock gate to 8/8. ----------
        warm_sb = rpool.tile([128, 640], BF16)
        nc.vector.memset(warm_sb[:, :], 0.0)
        warm_ps = l1ps.tile([128, 512], F32, tag="ps1", name="warm_ps")
        for _ in range(WARM):
            nc.tensor.matmul(
                warm_ps[:, :], lhsT=warm_sb[:, 0:128], rhs=warm_sb[:, 0:512],
                start=True, stop=True,
            )

        # onehot over embedding rows, bf16 [10, BL]: onehot[r, b] = (scene[b] == r)
        oh16_sb = rpool.tile([10, BL], BF16)
        nc.vector.tensor_scalar(
            out=oh16_sb[:, :], in0=srow10_sb[:, :],
            scalar1=io10_sb[:, 0:1], scalar2=None, op0=ALU.is_equal,
        )

        # ---- routing, reversed layout: psg[j, b] = sum_d sflat[d, j] x[d, b]
        # + sett[scene_b, j], computed as bf16 hi/lo three-term sums. ------
        psg = l2ps.tile([128, BL], F32, tag="ps2", name="psg")
        for kt in range(KT):
            nc.tensor.matmul(
                psg[0:EN, :], lhsT=sfh_sb[:, kt, :], rhs=xfull_sb[:, kt, :],
                start=(kt == 0), stop=False,
            )
            nc.tensor.matmul(
                psg[0:EN, :], lhsT=sfl_sb[:, kt, :], rhs=xfull_sb[:, kt, :],
                start=False, stop=False,
            )
            nc.tensor.matmul(
                psg[0:EN, :], lhsT=sfh_sb[:, kt, :], rhs=xl_sb[:, kt, :],
                start=False, stop=False,
            )
        nc.tensor.matmul(
            psg[0:EN, :], lhsT=setth_sb[:, :], rhs=oh16_sb[:, :],
            start=False, stop=False,
        )
        nc.tensor.matmul(
            psg[0:EN, :], lhsT=settl_sb[:, :], rhs=oh16_sb[:, :],
            start=False, stop=True,
        )
        gsb = rpool.tile([EN, BL], F32)
        nc.scalar.copy(gsb[:, :], psg[0:EN, :])

        def layer1(e, w18, w1lhs):
            """hT[f, b] = relu(sum_d W1[d, f] * xT[d, b] + b1[f]); fp8
            DoubleRow covers k rows [0, FP8_K), bf16 the rest."""
            ht_sb = htpool.tile([128, KT2, BL], BF16, tag="ht")
            for m in range(MT1):
                ps = l1ps.tile([128, BL], F32, tag="ps1")
                nc.tensor.matmul(
                    ps[:, :],
                    lhsT=w18[:, 0:KT8, bass.ts(m, 128)],
                    rhs=xq8_sb[:, 0:KT8, :],
                    start=True, stop=False,
                    perf_mode=DR,
                )
                for kt in range(KTB):
                    nc.tensor.matmul(
                        ps[:, :],
                        lhsT=w1lhs(m, kt),
                        rhs=xfull_sb[:, KT8 + kt, :],
                        start=False, stop=(kt == KTB - 1),
                    )
                bias1 = b1_sb[:, e * MT1 + m : e * MT1 + m + 1] if has_b1 else 0.0
                nc.scalar.activation(ht_sb[:, m, :], ps[:, :], AF.Relu, bias=bias1)
            return ht_sb

        def transposes():
            """[49, BL] routing result -> four [128, 49] b-tiles via PE."""
            psr = []
            for t in range(NB):
                psr_t = l1ps.tile([128, EN], F32, tag="ps1", name=f"psr{t}")
                nc.tensor.matmul(
                    psr_t[:, :], lhsT=gsb[:, bass.ts(t, 128)], rhs=ident_sb[:, :],
                    is_transpose=True,
                )
                psr.append(psr_t)
            return psr

        def routing_chain(psr):
            """Gate computation, fused over all 4 b-tiles ([128, 4*49])."""
            NE = NB * E  # 28
            gp = rpool.tile([128, NB * EN], F32)
            for t in range(NB):
                nc.scalar.copy(gp[:, bass.ts(t, EN)], psr[t][:, :])
            gp4 = gp.rearrange("p (t e s) -> p (t e) s", s=NS, e=E)
            eex = rpool.tile([128, NB * EN], F32)
            nc.scalar.activation(eex[:, :], gp[:, :], AF.Exp)
            z = rpool.tile([128, NE], F32)
            nc.vector.tensor_reduce(out=z[:, :], in_=eex.rearrange("p (t e s) -> p (t e) s", s=NS, e=E), axis=AX.X, op=ALU.add)
            logz = rpool.tile([128, NE], F32)
            nc.scalar.activation(logz[:, :], z[:, :], AF.Ln)
            sg = rpool.tile([128, NE], F32)
            nc.vector.tensor_reduce(out=sg[:, :], in_=gp4, axis=AX.X, op=ALU.add)
            q = rpool.tile([128, NE], F32)
            nc.vector.scalar_tensor_tensor(
                out=q[:, :], in0=sg[:, :], scalar=1.0 / NS, in1=logz[:, :],
                op0=ALU.mult, op1=ALU.subtract,
            )
            oh = rpool.tile([128, NB * EN], F32)
            nc.vector.tensor_tensor(out=oh[:, :], in0=io7_sb[:, :], in1=scolr_sb[:, :], op=ALU.is_equal)
            gsel = rpool.tile([128, NB * EN], F32)
            nc.vector.tensor_tensor(out=gsel[:, :], in0=gp[:, :], in1=oh[:, :], op=ALU.mult)
            s1s = rpool.tile([128, NE], F32)
            nc.vector.tensor_reduce(out=s1s[:, :], in_=gsel.rearrange("p (t e s) -> p (t e) s", s=NS, e=E), axis=AX.X, op=ALU.add)
            score1 = rpool.tile([128, NE], F32)
            nc.vector.tensor_tensor(out=score1[:, :], in0=s1s[:, :], in1=logz[:, :], op=ALU.subtract)

            lg = rpool.tile([128, NE], F32)
            nc.scalar.activation(lg[:, :], score1[:, :], AF.Exp)     # G at scene, in (0,1)
            el = rpool.tile([128, NE], F32)
            nc.scalar.activation(el[:, :], lg[:, :], AF.Exp)         # softmax numerator
            # per-b-tile scalars ([128,1]) for the reductions' broadcasts
            ssum = rpool.tile([128, NB], F32)
            rs = rpool.tile([128, NB], F32)
            m1 = rpool.tile([128, NB], F32)
            m2 = rpool.tile([128, NB], F32)
            k1 = rpool.tile([128, NE], F32)
            k2 = rpool.tile([128, NE], F32)
            g0 = rpool.tile([128, NE], F32)
            el3 = el.rearrange("p (t e) -> p t e", e=E)
            sc3 = score1.rearrange("p (t e) -> p t e", e=E)
            q3 = q.rearrange("p (t e) -> p t e", e=E)
            nc.vector.tensor_reduce(out=ssum[:, :], in_=el3, axis=AX.X, op=ALU.add)
            nc.vector.reciprocal(rs[:, :], ssum[:, :])
            nc.vector.tensor_reduce(out=m1[:, :], in_=sc3, axis=AX.X, op=ALU.min)
            nc.vector.tensor_reduce(out=m2[:, :], in_=q3, axis=AX.X, op=ALU.min)
            for t in range(NB):
                nc.vector.tensor_scalar(
                    out=k1[:, bass.ts(t, E)], in0=score1[:, bass.ts(t, E)],
                    scalar1=m1[:, t : t + 1], scalar2=None, op0=ALU.is_equal,
                )
                nc.vector.tensor_scalar(
                    out=k2[:, bass.ts(t, E)], in0=q[:, bass.ts(t, E)],
                    scalar1=m2[:, t : t + 1], scalar2=None, op0=ALU.is_equal,
                )
                nc.vector.tensor_scalar(
                    out=g0[:, bass.ts(t, E)], in0=el[:, bass.ts(t, E)],
                    scalar1=rs[:, t : t + 1], scalar2=None, op0=ALU.mult,
                )
            kill = rpool.tile([128, NE], F32)
            nc.vector.tensor_tensor(out=kill[:, :], in0=k1[:, :], in1=k2[:, :], op=ALU.mult)
            sel = rpool.tile([128, NE], F32)
            nc.vector.tensor_scalar(
                out=sel[:, :], in0=kill[:, :], scalar1=-1.0, scalar2=1.0,
                op0=ALU.mult, op1=ALU.add,
            )
            gate_flat = gate_sb.rearrange("p t e -> p (t e)")
            nc.vector.tensor_tensor(out=gate_flat[:, :], in0=g0[:, :], in1=sel[:, :], op=ALU.mult)

        def layer2(e, ht_sb, w2_sb):
            """out[b, o] = relu(sum_h hT[h, b] * W2[h, o] + b2[o]), gated and
            accumulated into acc_sb; last expert streams the result out."""
            last = e == E - 1
            for mb in range(NB):
                for no in range(NO):
                    ps2 = l2ps.tile([128, 512], F32, tag="ps2")
                    for kt in range(KT2):
                        nc.tensor.matmul(
                            ps2[:, :],
                            lhsT=ht_sb[:, kt, bass.ts(mb, 128)],
                            rhs=w2_sb[:, kt, bass.ts(no, 512)],
                            start=(kt == 0),
                            stop=(kt == KT2 - 1 and not has_b2),
                        )
                    if has_b2:
                        nc.tensor.matmul(
                            ps2[:, :],
                            lhsT=ones_sb[:, :],
                            rhs=b2_sb[:, e * H2 + no * 512 : e * H2 + (no + 1) * 512],
                            start=False, stop=True,
                        )
                    gcol = gate_sb[:, mb, e : e + 1]
                    if e == 0:
                        nc.scalar.activation(
                            acc_sb[:, mb, bass.ts(no, 512)], ps2[:, :], AF.Relu, scale=gcol
                        )
                    else:
                        tmp = tmppool.tile([128, 512], F32, tag="tmp")
                        nc.scalar.activation(tmp[:, :], ps2[:, :], AF.Relu, scale=gcol)
                        nc.vector.tensor_tensor(
                            out=acc_sb[:, mb, bass.ts(no, 512)],
                            in0=acc_sb[:, mb, bass.ts(no, 512)],
                            in1=tmp[:, :], op=ALU.add,
                        )
                    # Last batch tile of the last expert: store each 512-col
                    # half as soon as its add lands, shortening the tail.
                    if last and mb == NB - 1:
                        nc.sync.dma_start(
                            aps["out"].rearrange("(t p) o -> p t o", p=128)[
                                :, mb, bass.ts(no, 512)
                            ],
                            acc_sb[:, mb, bass.ts(no, 512)],
                        )
                if last and mb < NB - 1:
                    nc.sync.dma_start(
                        aps["out"].rearrange("(t p) o -> p t o", p=128)[:, mb, :],
                        acc_sb[:, mb, :],
                    )

        # ---- expert 0: L1 (while routing result waits), transpose+gate, L2
        ht_sb = layer1(0, w18_sb, w1lhs)
        psr = transposes()
        routing_chain(psr)
        rpool.release()
        nw18, nw1lhs = dma_w1(1)
        layer2(0, ht_sb, w2_sb)

        for e in range(1, E):
            w18_sb, w1lhs = nw18, nw1lhs
            w2_sb = dma_w2(e)
            ht_sb = layer1(e, w18_sb, w1lhs)
            if e < E - 1:
                nw18, nw1lhs = dma_w1(e + 1)
            layer2(e, ht_sb, w2_sb)


def build(has_b1, has_b2):
    """Build + schedule + compile the Bass program. Returns nc."""
    nc = bacc.Bacc("TRN2", target_bir_lowering=False, debug=False)
    aps = {}
    aps["xh"] = nc.dram_tensor("xh", [128, KT * BL], BF16, kind="ExternalInput").ap()
    aps["xl"] = nc.dram_tensor("xl", [128, KT * BL], BF16, kind="ExternalInput").ap()
    aps["xT8"] = nc.dram_tensor("xT8", [128, KT8 * BL], FP8, kind="ExternalInput").ap()
    aps["w18"] = nc.dram_tensor("w18", [E, 128, KT8 * H1], FP8, kind="ExternalInput").ap()
    aps["w1b"] = nc.dram_tensor("w1b", [E, 128, KTB * H1], BF16, kind="ExternalInput").ap()
    aps["w2"] = nc.dram_tensor("w2", [E, 128, KT2 * H2], BF16, kind="ExternalInput").ap()
    if has_b1:
        aps["b1t"] = nc.dram_tensor("b1t", [128, E * MT1], F32, kind="ExternalInput").ap()
    if has_b2:
        aps["b2f"] = nc.dram_tensor("b2f", [1, E * H2], BF16, kind="ExternalInput").ap()
    aps["sfh"] = nc.dram_tensor("sfh", [128, KT * EN], BF16, kind="ExternalInput").ap()
    aps["sfl"] = nc.dram_tensor("sfl", [128, KT * EN], BF16, kind="ExternalInput").ap()
    aps["setth"] = nc.dram_tensor("setth", [10, EN], BF16, kind="ExternalInput").ap()
    aps["settl"] = nc.dram_tensor("settl", [10, EN], BF16, kind="ExternalInput").ap()
    aps["ident"] = nc.dram_tensor("ident", [EN, EN], F32, kind="ExternalInput").ap()
    aps["scol_rep"] = nc.dram_tensor("scol_rep", [128, NB * EN], F32, kind="ExternalInput").ap()
    aps["srow"] = nc.dram_tensor("srow", [1, BL], F32, kind="ExternalInput").ap()
    aps["iota7"] = nc.dram_tensor("iota7", [1, NB * EN], F32, kind="ExternalInput").ap()
    aps["iota10"] = nc.dram_tensor("iota10", [10, 1], F32, kind="ExternalInput").ap()
    aps["out"] = nc.dram_tensor("out", [BL, H2], F32, kind="ExternalOutput").ap()

    with tile.TileContext(nc) as tc:
        _emit_kernel(tc, aps, has_b1, has_b2)
    nc.compile()
    return nc


def make_in_maps(inputs):
    """Host-side layout prep + batch sharding. Returns (in_maps, has_b1, has_b2)."""
    x = np.ascontiguousarray(np.asarray(inputs["x"], dtype=np.float32))
    scene = np.asarray(inputs["scene"]).astype(np.int64)
    W1 = np.asarray(inputs["W1"], dtype=np.float32)
    b1 = np.asarray(inputs["b1"], dtype=np.float32)
    W2 = np.asarray(inputs["W2"], dtype=np.float32)
    b2 = np.asarray(inputs["b2"], dtype=np.float32)
    S = np.asarray(inputs["S"], dtype=np.float32)
    scene_emb = np.asarray(inputs["scene_emb"], dtype=np.float32)

    has_b1 = bool(np.any(b1))
    has_b2 = bool(np.any(b2))

    def pmaj(a):
        """[T*128, F] -> partition-major [128, T*F]."""
        t = a.shape[0] // 128
        return np.ascontiguousarray(
            a.reshape(t, 128, a.shape[1]).transpose(1, 0, 2).reshape(128, -1)
        )

    def hilo(a):
        h = a.astype(NP_BF16)
        l = (a - h.astype(np.float32)).astype(NP_BF16)
        return np.ascontiguousarray(h), np.ascontiguousarray(l)

    w18 = np.stack([pmaj(W1[e, :FP8_K, :].astype(NP_FP8)) for e in range(E)])
    w1b = np.stack([pmaj(W1[e, FP8_K:, :].astype(NP_BF16)) for e in range(E)])
    w2b = np.stack([pmaj(W2[e].astype(NP_BF16)) for e in range(E)])
    sflat = np.ascontiguousarray(S[:, :D, :].transpose(1, 2, 0).reshape(D, EN))
    sfh, sfl = hilo(sflat)
    sfh, sfl = pmaj(sfh), pmaj(sfl)
    sett = np.ascontiguousarray(
        np.einsum("rm,sme->res", scene_emb, S[:, D:, :]).reshape(scene_emb.shape[0], EN)
    )
    setth, settl = hilo(sett)
    iota7 = np.tile(np.arange(EN, dtype=np.float32) % NS, NB).reshape(1, NB * EN)
    iota10 = np.arange(10, dtype=np.float32).reshape(10, 1)
    ident = np.eye(EN, dtype=np.float32)
    shared = {
        "w18": w18, "w1b": w1b, "w2": w2b, "sfh": sfh, "sfl": sfl,
        "setth": setth, "settl": settl, "ident": ident,
        "iota7": iota7, "iota10": iota10,
    }
    if has_b1:
        shared["b1t"] = np.ascontiguousarray(
            b1.reshape(E, MT1, 128).transpose(2, 0, 1).reshape(128, E * MT1)
        )
    if has_b2:
        shared["b2f"] = np.ascontiguousarray(b2.astype(NP_BF16).reshape(1, E * H2))

    in_maps = []
    for c in range(N_CORES):
        xs = x[c * BL : (c + 1) * BL]
        sc = scene[c * BL : (c + 1) * BL]
        xT = np.ascontiguousarray(xs.T)
        m = dict(shared)
        xhh, xll = hilo(xT)
        m["xh"], m["xl"] = pmaj(xhh), pmaj(xll)
        m["xT8"] = pmaj(np.ascontiguousarray(xT[:FP8_K].astype(NP_FP8)))
        scol = sc.reshape(NB, 128).T.astype(np.float32)          # [128, NB]
        m["scol_rep"] = np.ascontiguousarray(
            np.repeat(scol[:, :, None], EN, axis=2).reshape(128, NB * EN)
        )
        m["srow"] = np.ascontiguousarray(sc.astype(np.float32).reshape(1, BL))
        in_maps.append(m)
    return in_maps, has_b1, has_b2


_NC_CACHE = {}


def get_compiled(has_b1, has_b2):
    key = (has_b1, has_b2)
    if key not in _NC_CACHE:
        _NC_CACHE[key] = build(has_b1, has_b2)
    return _NC_CACHE[key]


def run(inputs, trace=False, **kwargs):
    """Run on hardware; returns (full_output, BassKernelResults)."""
    in_maps, has_b1, has_b2 = make_in_maps(inputs)
    nc = get_compiled(has_b1, has_b2)
    res = run_bass_kernel_spmd(nc, in_maps, core_ids=list(range(N_CORES)), trace=trace, **kwargs)
    parts = [res.results[c]["out"] for c in range(N_CORES)]
    out = np.concatenate(parts, axis=0).astype(np.float32)
    full = np.ascontiguousarray(np.broadcast_to(out[None], (T, B, H2)))
    return full, res


def kernel(**inputs):
    full, _ = run(inputs, trace=False)
    return full
